# revision 1
# baseline (speedup 1.0000x reference)
"""GAT layer (nn_GATLayer) on 8 Trainium2 NeuronCores via Bass/Tile.

Reference computation (N=8192, F=512, D=64):
    z = features @ W                      # [N, D]
    s = z @ a_self; t = z @ a_neigh       # [N, 1]
    e[i,j] = leakyrelu(s[i] + t[j], 0.2)
    attention = softmax(e + mask(A), axis=1)   # mask: -1e12 where A<=0
    h = attention @ z                     # [N, D]

Row-sharded across 8 cores (1024 attention rows each), two launches with a
host-side gather between them (replaces an on-device AllGather whose
rendezvous barrier alone costs ~47 us).

Key algebra: with e = s_i + t_j, q_j = exp(.8 t_j), p_i = exp(.8 s_i),
    exp(leakyrelu(e)) = exp(.2 t_j) * exp(.2 s_i) * max(p_i q_j, 1).
The exp(.2 s_i) column factor cancels between softmax numerator and
denominator and is dropped; exp(.2 t_j) is folded into the stationary
z' = [z | 1] * exp(.2 t_j) in launch A.  Per-tile mask weight is then
    ea = A * max(pq, 1)
and the crucial split (scheme Y/Z below)
    ea = A + relu(pq - 1) * A
lets the raw A tile stream straight from DMA into the PE (zero elementwise
cost) with the relu term as a second moving operand into the SAME psum
accumulator + stationary.  relu(pq - 1) is ONE scalar-engine activation:
Relu(p3 * eq_j - 1) via the per-partition `scale` operand.

Launch A (small): each core computes z (bf16 one-pass + W-lo correction),
t, s for its own 1024 rows; ships z' pre-scaled/cast to f16, eq = exp(.8t)
f16, s row f32.

Launch B (main): per j-chunk (128 j's x 1024 i's), one of three schemes
balances the elementwise mask work across engines under the A-DMA roofline:
  X: m  = max(p3*eq_j, 1)        (DVE tensor_scalar, 1 op)
     ea = m * A                  (DVE tensor_tensor) -> 1 moving stream
  Y: r  = Relu(p3*eq_j - 1)      (ACT activation, 1 op)
     g  = r * A                  (DVE tensor_tensor) -> 2 moving streams
  W/Z: same as X/Y but the tensor_tensor runs on GpSimd.
A-tiles arrive as 1 MB DMAs (4 chunks each) alternating sync/scalar HWDGE
queues; A ships as float16 0/1 (exact), host-prepacked to the on-chip
layout [128, JC*1024].
"""

import sys

sys.path.insert(0, "/opt/trn_rl_repo")

import numpy as np

N, F, D = 8192, 512, 64
NCORES = 8
R = N // NCORES          # rows per core (1024)
JC = N // 128            # j-chunks (64)
DP = D + 1               # z' | et2  (65)
ZW = 80                  # padded z' width (80 f16 = 160B rows)
ALPHA = 0.2
CPD = 4                  # chunks per A-DMA (4 * 256KB = 1MB)

# per-16-chunk scheme pattern (all single PE stream):
#   'V' = DVE ts + DVE tt (V chunks come in adjacent pairs -> batched tt)
#   'A' = ACT relu+exp pair      'G' = DVE ts + GpSimd tt
SCHEME16 = ['A', 'V', 'G', 'V', 'A', 'V', 'V', 'A',
            'V', 'V', 'A', 'G', 'V', 'A', 'V', 'V']

_CACHE = {}


def _build_launch_a():
    """Per-core z' = [z|1]*exp(.2 t) f16, eq = exp(.8 t) f16, s row f32."""
    import concourse.bacc as bacc
    import concourse.tile as tile
    from concourse import mybir
    from concourse.masks import make_identity

    f32 = mybir.dt.float32
    f16 = mybir.dt.float16
    bf16 = mybir.dt.bfloat16
    Alu = mybir.AluOpType
    Act = mybir.ActivationFunctionType

    nc = bacc.Bacc("TRN2", target_bir_lowering=False, debug=False, num_devices=NCORES)

    feat_t = nc.dram_tensor("feat_t", [F, R], bf16, kind="ExternalInput")
    w_in = nc.dram_tensor("w", [F, D], f32, kind="ExternalInput")
    a_self = nc.dram_tensor("a_self", [1, D], f32, kind="ExternalInput")
    a_neigh = nc.dram_tensor("a_neigh", [1, D], f32, kind="ExternalInput")
    za_out = nc.dram_tensor("za", [R, ZW], f16, kind="ExternalOutput")
    s_out = nc.dram_tensor("s", [1, R], f32, kind="ExternalOutput")
    eq_out = nc.dram_tensor("eq", [128, R // 128], f32, kind="ExternalOutput")
    t_out = nc.dram_tensor("t", [128, R // 128], f32, kind="ExternalOutput")

    IB = R // 128  # 8 row-blocks per core

    with tile.TileContext(nc) as tc:
        with (
            tc.tile_pool(name="sb", bufs=1) as cst,
            tc.tile_pool(name="ps", bufs=2, space="PSUM") as ps,
        ):
            ft = cst.tile([128, 4 * R], bf16)
            for c in range(4):
                nc.sync.dma_start(out=ft[:, c * R:(c + 1) * R],
                                  in_=feat_t[c * 128:(c + 1) * 128, :])
            w_sb = cst.tile([128, 4 * D], f32)
            for c in range(4):
                nc.scalar.dma_start(out=w_sb[:, c * D:(c + 1) * D],
                                    in_=w_in[c * 128:(c + 1) * 128, :])
            asr = cst.tile([1, D], f32)
            nc.scalar.dma_start(out=asr[:], in_=a_self[:])
            anr = cst.tile([1, D], f32)
            nc.scalar.dma_start(out=anr[:], in_=a_neigh[:])
            ones1 = cst.tile([1, 128], f32)
            nc.vector.memset(ones1[:], 1.0)

            # broadcast a_neigh / a_self across 128 partitions via PE
            pan = ps.tile([128, 2 * D], f32, tag="pro")
            nc.tensor.matmul(pan[:, 0:D], ones1[:], anr[:], start=True, stop=True)
            nc.tensor.matmul(pan[:, D:2 * D], ones1[:], asr[:], start=True, stop=True)
            anb = cst.tile([128, 2 * D], f32)
            nc.vector.tensor_copy(anb[:], pan[:])

            # W in bf16 hi + lo correction: z = f_bf16 @ wh + f_bf16 @ wl
            wh = cst.tile([128, 4 * D], bf16)
            nc.vector.tensor_copy(wh[:], w_sb[:])
            wl = cst.tile([128, 4 * D], bf16)
            nc.vector.tensor_tensor(wl[:], w_sb[:], wh[:], Alu.subtract)

            zsc = cst.tile([128, IB, D], f32)
            for ib in range(IB):
                psz = ps.tile([128, D], f32, tag="pro")
                first = True
                for wa in (wh, wl):
                    for c in range(4):
                        nc.tensor.matmul(
                            psz[:],
                            ft[:, c * R + ib * 128: c * R + (ib + 1) * 128],
                            wa[:, c * D:(c + 1) * D],
                            start=first, stop=(wa is wl and c == 3),
                        )
                        first = False
                nc.vector.tensor_copy(zsc[:, ib], psz[:])

            # t = z @ a_neigh, s = z @ a_self  (free-axis reduces)
            tscr = cst.tile([128, IB, D], f32)
            for ib in range(IB):
                nc.vector.tensor_tensor(tscr[:, ib], zsc[:, ib], anb[:, 0:D], Alu.mult)
            t_sb = cst.tile([128, IB], f32)
            nc.vector.tensor_reduce(t_sb[:], tscr[:], mybir.AxisListType.X, Alu.add)
            for ib in range(IB):
                nc.vector.tensor_tensor(tscr[:, ib], zsc[:, ib], anb[:, D:2 * D], Alu.mult)
            s_sb = cst.tile([128, IB], f32)
            nc.vector.tensor_reduce(s_sb[:], tscr[:], mybir.AxisListType.X, Alu.add)

            # s row: transpose [128, IB] -> [IB, 128] -> flat [1, R]
            ident = cst.tile([128, 128], f32)
            make_identity(nc, ident[:])
            pst = ps.tile([IB, 128], f32, tag="pro")
            nc.tensor.transpose(pst[:], s_sb[:], ident[:])
            st_sb = cst.tile([IB, 128], f32)
            nc.vector.tensor_copy(st_sb[:], pst[:])
            nc.sync.dma_start(
                out=s_out[:].rearrange("o (p c) -> (o p) c", p=IB), in_=st_sb[:])

            # et2 = exp(.2 t) f32; eq = exp(.8 t) f16 (shipped)
            et2 = cst.tile([128, IB], f32)
            nc.scalar.activation(et2[:], t_sb[:], Act.Exp, scale=ALPHA)
            eq_sb = cst.tile([128, IB], f32)
            nc.scalar.activation(eq_sb[:], t_sb[:], Act.Exp, scale=1.0 - ALPHA)
            nc.sync.dma_start(out=eq_out[:], in_=eq_sb[:])
            nc.sync.dma_start(out=t_out[:], in_=t_sb[:])

            # z' = [z * et2 | et2 | pad] in f16
            za_sb = cst.tile([128, IB, ZW], f16)
            nc.vector.memset(za_sb[:], 0.0)
            for ib in range(IB):
                nc.vector.tensor_scalar_mul(
                    za_sb[:, ib, 0:D], zsc[:, ib], et2[:, ib:ib + 1])
            nc.vector.tensor_copy(za_sb[:, :, D], et2[:])
            nc.sync.dma_start(
                out=za_out[:].rearrange("(c p) d -> p c d", p=128), in_=za_sb[:])

    nc.compile()
    return nc


def _build_launch_b():
    import concourse.bacc as bacc
    import concourse.tile as tile
    from concourse import mybir

    f32 = mybir.dt.float32
    f16 = mybir.dt.float16
    Alu = mybir.AluOpType
    Act = mybir.ActivationFunctionType

    nc = bacc.Bacc("TRN2", target_bir_lowering=False, debug=False, num_devices=NCORES)

    a_t = nc.dram_tensor("a_t", [128, JC * R], f16, kind="ExternalInput")
    zaf = nc.dram_tensor("zaf", [128, JC * ZW], f16, kind="ExternalInput")
    s_in = nc.dram_tensor("s", [1, R], f32, kind="ExternalInput")
    eq_in = nc.dram_tensor("eq", [128, JC], f32, kind="ExternalInput")
    t_in = nc.dram_tensor("t", [128, JC], f32, kind="ExternalInput")
    h_out = nc.dram_tensor("h", [R, D], f32, kind="ExternalOutput")

    schemes = [SCHEME16[jc % 16] for jc in range(JC)]

    with tile.TileContext(nc) as tc:
        with (
            tc.tile_pool(name="const", bufs=1) as cst,
            tc.tile_pool(name="ps_main", bufs=2, space="PSUM") as ps_main,
        ):
            # small inputs first: eq/t and s unblock the score chain early
            eq = cst.tile([128, JC], f32)
            nc.scalar.dma_start(out=eq[:], in_=eq_in[:])
            tt_sb = cst.tile([128, JC], f32)
            nc.scalar.dma_start(out=tt_sb[:], in_=t_in[:])
            s_row = cst.tile([1, R], f32)
            nc.scalar.dma_start(out=s_row[:], in_=s_in[:])
            zf = cst.tile([128, JC, ZW], f16)        # z', j-chunked
            nc.sync.dma_start(
                out=zf[:], in_=zaf[:].rearrange("p (c d) -> p c d", d=ZW))
            ones1 = cst.tile([1, 128], f32)
            nc.vector.memset(ones1[:], 1.0)

            # t8 = .8 t (bias for the ACT relu chunks)
            t8 = cst.tile([128, JC], f32)
            nc.scalar.activation(t8[:], tt_sb[:], Act.Identity,
                                 scale=1.0 - ALPHA)

            # s broadcast across partitions (PE); s_bcast f32 + p3 f16
            psb = ps_main.tile([128, R], f32, tag="hp", name="psb")
            for hh in range(2):
                nc.tensor.matmul(
                    psb[:, hh * 512:(hh + 1) * 512],
                    ones1[:],
                    s_row[0:1, hh * 512:(hh + 1) * 512],
                    start=True, stop=True,
                )
            s_bcast = cst.tile([128, R], f32)
            nc.vector.tensor_copy(s_bcast[:], psb[:])
            p3 = cst.tile([128, R], f16)
            nc.scalar.activation(p3[:], s_bcast[:], Act.Exp, scale=1.0 - ALPHA)

            # two H' accumulators: even/odd chunks accumulate separately
            hps = [ps_main.tile([DP, R], f32, tag="hp", name=f"hp{g}")
                   for g in range(2)]

            # ---- main loop over j-chunks, A arrives 4 chunks per DMA ----
            with (
                tc.tile_pool(name="a_pool", bufs=4) as a_pool,
                tc.tile_pool(name="work", bufs=8) as work,
            ):
                dma_engines = [nc.sync, nc.scalar]
                a_tiles = {}
                for jc in range(JC):
                    if jc % CPD == 0:
                        blk = jc // CPD
                        atile = a_pool.tile([128, CPD * R], f16, tag="at")
                        dma_engines[blk % 2].dma_start(
                            out=atile[:],
                            in_=a_t[:, blk * CPD * R:(blk + 1) * CPD * R])
                        a_tiles[blk] = atile
                    at = a_tiles[jc // CPD][:, (jc % CPD) * R:(jc % CPD + 1) * R]

                    sch = schemes[jc]
                    par = jc % 2
                    hp = hps[par]
                    start = jc in (0, 1)
                    stop = jc in (JC - 2, JC - 1)
                    zst = zf[:, jc, 0:DP]

                    if sch in ('V', 'G'):
                        m = work.tile([128, R], f16, tag="m")
                        nc.vector.tensor_scalar(
                            m[:], p3[:], eq[:, jc:jc + 1], 1.0,
                            Alu.mult, Alu.max)
                    else:  # ACT pair: u = relu(.8(s+t)); m = exp(u)
                        u = work.tile([128, R], f32, tag="u")
                        nc.scalar.activation(
                            u[:], s_bcast[:], Act.Relu,
                            bias=t8[:, jc:jc + 1], scale=1.0 - ALPHA)
                        m = work.tile([128, R], f16, tag="m")
                        nc.scalar.activation(m[:], u[:], Act.Exp)
                    ea = work.tile([128, R], f16, tag="ea")
                    eng = nc.gpsimd if sch == 'G' else nc.vector
                    eng.tensor_tensor(ea[:], m[:], at, Alu.mult)
                    ea_sl = ea[:]
                    for hh in range(2):
                        nc.tensor.matmul(
                            hp[:, hh * 512:(hh + 1) * 512],
                            zst, ea_sl[:, hh * 512:(hh + 1) * 512],
                            start=start, stop=stop,
                        )

            # ---- epilogue: transpose H', normalize, store ----
            with (
                tc.tile_pool(name="epi", bufs=2) as epi,
            ):
                from concourse.masks import make_identity
                h_sb = cst.tile([DP, R], f32)
                nc.vector.tensor_copy(h_sb[:], hps[0][:])
                nc.vector.tensor_tensor(h_sb[:], h_sb[:], hps[1][:], Alu.add)
                ident = cst.tile([DP, DP], f32)
                make_identity(nc, ident[:])
                for b in range(R // 128):
                    trp = ps_main.tile([128, DP], f32, tag="hp")
                    nc.tensor.transpose(
                        trp[:], h_sb[:, b * 128:(b + 1) * 128], ident[:])
                    rec = epi.tile([128, 1], f32, tag="rec")
                    nc.vector.reciprocal(rec[:], trp[:, D:DP])
                    hb = epi.tile([128, D], f32, tag="hb")
                    nc.vector.tensor_scalar_mul(hb[:], trp[:, 0:D], rec[:, 0:1])
                    nc.sync.dma_start(
                        out=h_out[b * 128:(b + 1) * 128, :], in_=hb[:])

    nc.compile()
    return nc


def _get_programs():
    if "a" not in _CACHE:
        _CACHE["a"] = _build_launch_a()
        _CACHE["b"] = _build_launch_b()
    return _CACHE["a"], _CACHE["b"]


def _mask_to_f16(block):
    """0/1 int mask -> float16 exactly, fast (bit pattern 0x3C00 = 1.0)."""
    bits = (block != 0).astype(np.uint16) * np.uint16(0x3C00)
    return bits.view(np.float16)


def prepare_inputs_a(features, W, a_self, a_neigh):
    features = np.asarray(features, dtype=np.float32)
    feat_bf = _f32_to_bf16(features)
    W = np.ascontiguousarray(np.asarray(W, dtype=np.float32))
    a_self_r = np.ascontiguousarray(np.asarray(a_self, dtype=np.float32).reshape(1, D))
    a_neigh_r = np.ascontiguousarray(np.asarray(a_neigh, dtype=np.float32).reshape(1, D))
    in_a = []
    for k in range(NCORES):
        rows = slice(k * R, (k + 1) * R)
        in_a.append({
            "feat_t": np.ascontiguousarray(feat_bf[rows, :].T),
            "w": W,
            "a_self": a_self_r,
            "a_neigh": a_neigh_r,
        })
    return in_a


def _f32_to_bf16(x):
    import ml_dtypes
    return x.astype(ml_dtypes.bfloat16)


def prepare_inputs_b(A, res_a):
    za_rows = np.concatenate([res_a[k]["za"] for k in range(NCORES)], axis=0)
    # B-layout: zaf[p, c*ZW+d] = z'[c*128+p, d]
    zaf = np.ascontiguousarray(
        za_rows.reshape(JC, 128, ZW).transpose(1, 0, 2).reshape(128, JC * ZW))
    eq_full = np.ascontiguousarray(
        np.concatenate([res_a[k]["eq"] for k in range(NCORES)], axis=1))
    t_full = np.ascontiguousarray(
        np.concatenate([res_a[k]["t"] for k in range(NCORES)], axis=1))
    in_b = []
    for k in range(NCORES):
        rows = slice(k * R, (k + 1) * R)
        blk = _mask_to_f16(np.asarray(A[rows, :]))      # [R, N] 0/1 f16
        # at[p, jc*R + i] = A[k*R + i, jc*128 + p]
        at = np.ascontiguousarray(
            blk.reshape(R, JC, 128).transpose(2, 1, 0).reshape(128, JC * R))
        in_b.append({
            "a_t": at,
            "zaf": zaf,
            "s": res_a[k]["s"],
            "eq": eq_full,
            "t": t_full,
        })
    return in_b


def kernel(features, A, W, a_self, a_neigh):
    from concourse.bass_utils import run_bass_kernel_spmd

    nca, ncb = _get_programs()
    in_a = prepare_inputs_a(features, W, a_self, a_neigh)
    res_a = run_bass_kernel_spmd(nca, in_a, list(range(NCORES))).results
    in_b = prepare_inputs_b(A, res_a)
    res_b = run_bass_kernel_spmd(ncb, in_b, list(range(NCORES))).results
    h = np.concatenate([res_b[k]["h"] for k in range(NCORES)], axis=0)
    return h.astype(np.float32)



# revision 9
# speedup vs baseline: 1.7516x; 1.7516x over previous
"""GAT layer (nn_GATLayer) on 8 Trainium2 NeuronCores via Bass/Tile.

Reference computation (N=8192, F=512, D=64):
    z = features @ W                      # [N, D]
    s = z @ a_self; t = z @ a_neigh       # [N, 1]
    e[i,j] = leakyrelu(s[i] + t[j], 0.2)
    attention = softmax(e + mask(A), axis=1)   # mask: -1e12 where A<=0
    h = attention @ z                     # [N, D]

Strategy (v2, "sorted-zone" kernel):
  Row-shard i across 8 cores (1024 rows each).  The attention weight
  factor is exp(lrelu(u)) = e^{.2u} * max(e^{.8u}, 1), u = s_i + t_j.
  The e^{.2s_i} factor cancels in softmax; e^{.2t_j} folds into the
  stationary z' = [z|1] * e^{.2t}.  Remaining per-pair factor:
      m_ij = max(p_i q_j, 1),  p = e^{.8s}, q = e^{.8t}.
  KEY: sort i by s (within each core) and j by t (globally; both are
  host-side relabelings, softmax is order-invariant).  For a j-chunk
  pair with t in [tmin, tmax]:
     i < c_lo  (s_i < -tmax)  =>  u < 0  =>  m = 1   exactly
     i >= c_hi (s_i >= -tmin) =>  u >= 0 =>  m = p q exactly (separable!)
  so only the narrow kink band [c_lo, c_hi) (~7% of elements) needs
  per-element m.  The LO zone streams RAW A into the PE (stationary z'),
  the HI zone streams RAW A with stationary z'*q/K (post-scaled by p*K
  per-row in the epilogue), and the band uses ea = max(p q, 1) * A.
  A ships as fp8 (0/1 exact, half the DMA of f16) and all main matmuls
  run in fp8 DoubleRow mode (2 j-subblocks contracted per pass, 0.5
  cycles/output-col).

  Launch A computes z^T per core (bf16 hi+lo 2-pass); s, t, sorting,
  zone bounds, and all packing happen on the host between launches.
  Launch B is compiled per zone-bound tuple (data-dependent constants;
  cached after first call).
"""

import sys

sys.path.insert(0, "/opt/trn_rl_repo")

import numpy as np
import ml_dtypes

N, F, D = 8192, 512, 64
NCORES = 8
R = N // NCORES          # rows (i) per core: 1024
JC = N // 128            # j-chunks of 128: 64
PAIR = 2                 # j-chunks per DoubleRow group
G = JC // PAIR           # groups: 32
DP = D + 1               # z' width: [z | 1] scaled
ZW8 = 72                 # padded fp8 stationary row (65 -> 72)
IB = R // 128            # i-blocks per core: 8
ALPHA = 0.2
KSC = 2.0                # zq pre-scale: zq = z' * q / KSC, post *KSC
CPD = 4                  # chunks per A-DMA (4 * 128KB = 512KB fp8)

F8 = ml_dtypes.float8_e4m3
_CACHE = {}


# ----------------------------------------------------------------- launch A
def _build_launch_a():
    """Per-core z^T = (feat @ W)^T as f16 [D, R]; bf16 hi+lo two-pass."""
    import concourse.bacc as bacc
    import concourse.tile as tile
    from concourse import mybir

    f32 = mybir.dt.float32
    f16 = mybir.dt.float16
    bf16 = mybir.dt.bfloat16
    Act = mybir.ActivationFunctionType

    nc = bacc.Bacc("TRN2", target_bir_lowering=False, debug=False,
                   num_devices=NCORES)

    feat_t = nc.dram_tensor("feat_t", [F, R], bf16, kind="ExternalInput")
    wh_in = nc.dram_tensor("wh", [128, 4 * D], bf16, kind="ExternalInput")
    wl_in = nc.dram_tensor("wl", [128, 4 * D], bf16, kind="ExternalInput")
    zt_out = nc.dram_tensor("zt", [D, R], f16, kind="ExternalOutput")

    with tile.TileContext(nc) as tc:
        with (
            tc.tile_pool(name="sb", bufs=1) as cst,
            tc.tile_pool(name="ps", bufs=1, space="PSUM") as ps,
        ):
            ft = cst.tile([128, 4, R], bf16)
            engs = [nc.sync, nc.scalar]
            for c in range(4):
                engs[c % 2].dma_start(out=ft[:, c],
                                      in_=feat_t[c * 128:(c + 1) * 128, :])
            wh = cst.tile([128, 4 * D], bf16)
            nc.scalar.dma_start(out=wh[:], in_=wh_in[:])
            wl = cst.tile([128, 4 * D], bf16)
            nc.sync.dma_start(out=wl[:], in_=wl_in[:])

            psz = ps.tile([D, R], f32)
            for hh in range(2):
                first = True
                for wa in (wh, wl):
                    for c in range(4):
                        nc.tensor.matmul(
                            psz[:, hh * 512:(hh + 1) * 512],
                            wa[:, c * D:(c + 1) * D],
                            ft[:, c, hh * 512:(hh + 1) * 512],
                            start=first,
                            stop=(wa is wl and c == 3),
                        )
                        first = False
            zt_sb = cst.tile([D, R], f16)
            nc.scalar.activation(zt_sb[:], psz[:], Act.Copy)
            nc.sync.dma_start(out=zt_out[:], in_=zt_sb[:])

    nc.compile()
    return nc


# ----------------------------------------------------------------- launch B
def _build_launch_b(c_lo, c_hi):
    """Zoned attention kernel; c_lo/c_hi are per-group ints (len G)."""
    import concourse.bacc as bacc
    import concourse.tile as tile
    from concourse import mybir

    f32 = mybir.dt.float32
    f16 = mybir.dt.float16
    f8 = mybir.dt.float8e4
    Alu = mybir.AluOpType
    Act = mybir.ActivationFunctionType
    DR = mybir.MatmulPerfMode.DoubleRow

    nc = bacc.Bacc("TRN2", target_bir_lowering=False, debug=False,
                   num_devices=NCORES)

    a_t = nc.dram_tensor("a_t", [128, JC * R], f8, kind="ExternalInput")
    zpf_in = nc.dram_tensor("zpf", [128, JC * D], f8, kind="ExternalInput")
    zqf_in = nc.dram_tensor("zqf", [128, JC * D], f8, kind="ExternalInput")
    dpf_in = nc.dram_tensor("dpf", [128, JC * 16], f8, kind="ExternalInput")
    dqf_in = nc.dram_tensor("dqf", [128, JC * 16], f8, kind="ExternalInput")
    p3_in = nc.dram_tensor("p3", [128, R], f16, kind="ExternalInput")
    pscl_in = nc.dram_tensor("pscl", [128, IB], f32, kind="ExternalInput")
    eq_in = nc.dram_tensor("eqv", [128, JC], f32, kind="ExternalInput")
    h_out = nc.dram_tensor("h", [R, D], f32, kind="ExternalOutput")

    def segs(a, b):
        """Split [a,b) column range at the 512 psum-bank boundary."""
        if a >= b:
            return []
        if a < 512 < b:
            return [(a, 512), (512, b)]
        return [(a, b)]

    with tile.TileContext(nc) as tc:
        with (
            tc.tile_pool(name="const", bufs=1) as cst,
            tc.tile_pool(name="ps_acc", bufs=1, space="PSUM") as ps_acc,
            tc.tile_pool(name="ps_tr", bufs=2, space="PSUM") as ps_tr,
        ):
            zpf = cst.tile([128, JC, D], f8)
            nc.sync.dma_start(
                out=zpf[:], in_=zpf_in[:].rearrange("p (c d) -> p c d", d=D))
            zqf = cst.tile([128, JC, D], f8)
            nc.scalar.dma_start(
                out=zqf[:], in_=zqf_in[:].rearrange("p (c d) -> p c d", d=D))
            dpf = cst.tile([128, JC, 16], f8)
            nc.gpsimd.dma_start(
                out=dpf[:], in_=dpf_in[:].rearrange("p (c d) -> p c d", d=16))
            dqf = cst.tile([128, JC, 16], f8)
            nc.gpsimd.dma_start(
                out=dqf[:], in_=dqf_in[:].rearrange("p (c d) -> p c d", d=16))
            p3 = cst.tile([128, R], f16)
            nc.gpsimd.dma_start(out=p3[:], in_=p3_in[:])
            pscl = cst.tile([128, IB], f32)
            nc.gpsimd.dma_start(out=pscl[:], in_=pscl_in[:])
            eqv = cst.tile([128, JC], f32)
            nc.gpsimd.dma_start(out=eqv[:], in_=eq_in[:])

            ones512 = cst.tile([1, 512], f16)
            nc.vector.memset(ones512[:], 1.0)
            zrow64 = cst.tile([1, D], f16)
            nc.vector.memset(zrow64[:], 0.0)
            zrow16 = cst.tile([1, 16], f16)
            nc.vector.memset(zrow16[:], 0.0)

            # accumulators, zero-initialized via [1]-contraction matmuls
            acc0 = ps_acc.tile([D, R], f32, name="acc0")
            acc1 = ps_acc.tile([D, R], f32, name="acc1")
            dacc = ps_acc.tile([16, R], f32, name="dacc")
            for acc, zr in ((acc0, zrow64), (acc1, zrow64), (dacc, zrow16)):
                for hh in range(2):
                    nc.tensor.matmul(
                        acc[:, hh * 512:(hh + 1) * 512],
                        zr[:], ones512[:],
                        start=True, stop=False, skip_group_check=True,
                    )

            # ---- main loop over DoubleRow groups (2 j-chunks each) ----
            with (
                tc.tile_pool(name="a_pool", bufs=6) as a_pool,
                tc.tile_pool(name="work", bufs=8) as work,
            ):
                dma_engines = [nc.sync, nc.scalar]
                a_tiles = {}
                for g in range(G):
                    blk = (g * PAIR) // CPD
                    if (g * PAIR) % CPD == 0:
                        atile = a_pool.tile([128, CPD, R], f8, tag="at")
                        dma_engines[blk % 2].dma_start(
                            out=atile[:],
                            in_=a_t[:, blk * CPD * R:(blk + 1) * CPD * R]
                            .rearrange("p (c r) -> p c r", r=R))
                        a_tiles[blk] = atile
                    c0 = (g * PAIR) % CPD        # chunk offset in tile
                    at = a_tiles[blk]
                    lo, hi = int(c_lo[g]), int(c_hi[g])
                    W = hi - lo

                    zst = zpf[:, g * PAIR:(g + 1) * PAIR, :]
                    zqt = zqf[:, g * PAIR:(g + 1) * PAIR, :]
                    dpt = dpf[:, g * PAIR:(g + 1) * PAIR, :]
                    dqt = dqf[:, g * PAIR:(g + 1) * PAIR, :]

                    if W > 0:
                        # band: at <- max(p3*eq, 1) * at, in place
                        m2 = work.tile([128, PAIR, W], f16, tag="m")
                        for mem in range(PAIR):
                            nc.vector.tensor_scalar(
                                m2[:, mem], p3[:, lo:hi],
                                eqv[:, g * PAIR + mem:g * PAIR + mem + 1],
                                1.0, Alu.mult, Alu.max)
                        nc.vector.tensor_tensor(
                            at[:, c0:c0 + PAIR, lo:hi],
                            m2[:], at[:, c0:c0 + PAIR, lo:hi],
                            Alu.mult)
                    # LO+band zone: [0, hi) raw A (band cols now hold ea)
                    for (a, b) in segs(0, hi):
                        nc.tensor.matmul(
                            acc0[:, a:b], zst, at[:, c0:c0 + PAIR, a:b],
                            start=False, stop=False, perf_mode=DR,
                            skip_group_check=True)
                        nc.tensor.matmul(
                            dacc[:, a:b], dpt, at[:, c0:c0 + PAIR, a:b],
                            start=False, stop=False, perf_mode=DR,
                            skip_group_check=True)
                    # HI zone: [hi, R)
                    for (a, b) in segs(hi, R):
                        nc.tensor.matmul(
                            acc1[:, a:b], zqt, at[:, c0:c0 + PAIR, a:b],
                            start=False, stop=False, perf_mode=DR,
                            skip_group_check=True)
                        nc.tensor.matmul(
                            dacc[:, a:b], dqt, at[:, c0:c0 + PAIR, a:b],
                            start=False, stop=False, perf_mode=DR,
                            skip_group_check=True)

                # close the accumulation groups (sim bookkeeping)
                for acc, zr in ((acc0, zrow64), (acc1, zrow64), (dacc, zrow16)):
                    for hh in range(2):
                        nc.tensor.matmul(
                            acc[:, hh * 512:(hh + 1) * 512],
                            zr[:], ones512[:],
                            start=False, stop=True, skip_group_check=True,
                        )

            # ---- epilogue: h = (acc0 + K p acc1) / (den0 + K p den1) ----
            with tc.tile_pool(name="epi", bufs=3) as epi:
                from concourse.masks import make_identity
                h0 = cst.tile([D, R], f32)
                nc.scalar.activation(h0[:], acc0[:], Act.Copy)
                h1 = cst.tile([D, R], f32)
                nc.scalar.activation(h1[:], acc1[:], Act.Copy)
                hd = cst.tile([16, R], f32)
                nc.scalar.activation(hd[:], dacc[:], Act.Copy)
                ident = cst.tile([D, D], f32)
                make_identity(nc, ident[:])
                for b in range(IB):
                    tr0 = ps_tr.tile([128, D], f32, tag="tr")
                    nc.tensor.transpose(
                        tr0[:], h0[:, b * 128:(b + 1) * 128], ident[:])
                    tr1 = ps_tr.tile([128, D], f32, tag="tr")
                    nc.tensor.transpose(
                        tr1[:], h1[:, b * 128:(b + 1) * 128], ident[:])
                    trd = ps_tr.tile([128, 16], f32, tag="tr")
                    nc.tensor.transpose(
                        trd[:], hd[:, b * 128:(b + 1) * 128],
                        ident[0:16, 0:16])
                    t0s = epi.tile([128, D], f32, tag="t0")
                    nc.vector.tensor_copy(t0s[:], tr0[:])
                    hb = epi.tile([128, D], f32, tag="hb")
                    nc.vector.scalar_tensor_tensor(
                        hb[:], tr1[:], pscl[:, b:b + 1], t0s[:],
                        Alu.mult, Alu.add)
                    dts = epi.tile([128, 16], f32, tag="dt")
                    nc.vector.tensor_copy(dts[:], trd[:])
                    dcol = epi.tile([128, 1], f32, tag="dc")
                    nc.vector.scalar_tensor_tensor(
                        dcol[:], dts[:, 1:2], pscl[:, b:b + 1], dts[:, 0:1],
                        Alu.mult, Alu.add)
                    rec = epi.tile([128, 1], f32, tag="rec")
                    nc.vector.reciprocal(rec[:], dcol[:])
                    ho = epi.tile([128, D], f32, tag="ho")
                    nc.vector.tensor_scalar_mul(ho[:], hb[:], rec[:, 0:1])
                    nc.sync.dma_start(
                        out=h_out[b * 128:(b + 1) * 128, :], in_=ho[:])

    nc.compile()
    return nc


def _get_launch_a():
    if "a" not in _CACHE:
        _CACHE["a"] = _build_launch_a()
    return _CACHE["a"]


def _get_launch_b(c_lo, c_hi):
    key = ("b", tuple(c_lo), tuple(c_hi))
    if key not in _CACHE:
        _CACHE[key] = _build_launch_b(c_lo, c_hi)
    return _CACHE[key]


# ----------------------------------------------------------------- host side
def _f32_to_bf16(x):
    return x.astype(ml_dtypes.bfloat16)


def prepare_inputs_a(features, W):
    features = np.asarray(features, dtype=np.float32)
    feat_bf = _f32_to_bf16(features)
    W = np.asarray(W, dtype=np.float32)
    wh = _f32_to_bf16(W)
    wl = _f32_to_bf16(W - wh.astype(np.float32))
    # [F, D] -> [128, 4*D] chunked rows
    whp = np.ascontiguousarray(
        wh.reshape(4, 128, D).transpose(1, 0, 2).reshape(128, 4 * D))
    wlp = np.ascontiguousarray(
        wl.reshape(4, 128, D).transpose(1, 0, 2).reshape(128, 4 * D))
    in_a = []
    for k in range(NCORES):
        rows = slice(k * R, (k + 1) * R)
        in_a.append({
            "feat_t": np.ascontiguousarray(feat_bf[rows, :].T),
            "wh": whp,
            "wl": wlp,
        })
    return in_a


def _zone_bounds(s_sorted_cores, t_s):
    c_lo = np.empty(G, np.int64)
    c_hi = np.empty(G, np.int64)
    for g in range(G):
        tmin = t_s[g * 128 * PAIR]
        tmax = t_s[(g + 1) * 128 * PAIR - 1]
        lo, hi = R, 0
        for ss in s_sorted_cores:
            lo = min(lo, int(np.searchsorted(ss, -tmax)))
            hi = max(hi, int(np.searchsorted(ss, -tmin)))
        c_lo[g], c_hi[g] = lo, hi
    return c_lo, c_hi


def prepare_inputs_b(A, res_a, a_self, a_neigh):
    """Host: s/t from z, sorts, zone bounds, fp8 packing."""
    a_self = np.asarray(a_self, np.float32).reshape(D)
    a_neigh = np.asarray(a_neigh, np.float32).reshape(D)
    z = np.concatenate(
        [np.asarray(res_a[k]["zt"], np.float32).T for k in range(NCORES)], 0)
    s = z @ a_self
    t = z @ a_neigh

    jord = np.argsort(t)
    t_s = t[jord]
    zs = z[jord]
    et2 = np.exp(ALPHA * t_s).astype(np.float32)
    q = np.exp((1.0 - ALPHA) * t_s).astype(np.float32)
    zp = zs * et2[:, None]
    zq = zp * (q[:, None] / KSC)

    def pack(m, w):  # [N, w] f32 -> [128, JC*w] fp8
        m8 = np.ascontiguousarray(m).astype(F8)
        return np.ascontiguousarray(
            m8.reshape(JC, 128, w).transpose(1, 0, 2).reshape(128, JC * w))

    zpf = pack(zp, D)
    zqf = pack(zq, D)
    dp = np.zeros((N, 16), np.float32)
    dp[:, 0] = et2
    dq = np.zeros((N, 16), np.float32)
    dq[:, 1] = et2 * q / KSC
    dpf = pack(dp, 16)
    dqf = pack(dq, 16)
    eqv = np.ascontiguousarray(q.astype(np.float32).reshape(JC, 128).T)

    iord_cores, s_sorted_cores = [], []
    for k in range(NCORES):
        sk = s[k * R:(k + 1) * R]
        io = np.argsort(sk)
        iord_cores.append(io)
        s_sorted_cores.append(sk[io])
    c_lo, c_hi = _zone_bounds(s_sorted_cores, t_s)

    Ab = (np.asarray(A) != 0)
    one8_byte = np.array(1.0, F8).view(np.uint8)  # exact 1.0 bit pattern
    in_b = []
    for k in range(NCORES):
        rows = slice(k * R, (k + 1) * R)
        Bk = Ab[rows][iord_cores[k]][:, jord]
        a8 = (Bk.astype(np.uint8) * one8_byte).view(F8)
        at = np.ascontiguousarray(
            a8.reshape(R, JC, 128).transpose(2, 1, 0).reshape(128, JC * R))
        ss = s_sorted_cores[k].astype(np.float32)
        p3row = np.exp((1.0 - ALPHA) * ss).astype(np.float16)
        in_b.append({
            "a_t": at,
            "zpf": zpf,
            "zqf": zqf,
            "dpf": dpf,
            "dqf": dqf,
            "p3": np.ascontiguousarray(
                np.broadcast_to(p3row[None, :], (128, R))),
            "pscl": np.ascontiguousarray(
                (KSC * np.exp((1.0 - ALPHA) * ss)).astype(np.float32)
                .reshape(IB, 128).T),
            "eqv": eqv,
        })
    return in_b, c_lo, c_hi, iord_cores


def kernel_impl(features, A, W, a_self, a_neigh, trace_dirs=None):
    from concourse.bass_utils import run_bass_kernel_spmd

    times = {}
    nca = _get_launch_a()
    in_a = prepare_inputs_a(features, W)
    kw = {}
    if trace_dirs:
        kw = dict(trace=True, tmpdir=trace_dirs[0])
    ra = run_bass_kernel_spmd(nca, in_a, list(range(NCORES)), **kw)
    if trace_dirs:
        times["A"] = ra.exec_time_ns
    res_a = ra.results

    in_b, c_lo, c_hi, iord_cores = prepare_inputs_b(A, res_a, a_self, a_neigh)
    ncb = _get_launch_b(c_lo, c_hi)
    kw = {}
    if trace_dirs:
        kw = dict(trace=True, tmpdir=trace_dirs[1])
    rb = run_bass_kernel_spmd(ncb, in_b, list(range(NCORES)), **kw)
    if trace_dirs:
        times["B"] = rb.exec_time_ns
    res_b = rb.results

    h = np.empty((N, D), np.float32)
    for k in range(NCORES):
        hk = np.asarray(res_b[k]["h"], np.float32)
        blk = np.empty_like(hk)
        blk[iord_cores[k]] = hk
        h[k * R:(k + 1) * R] = blk
    return h, times


def kernel(features, A, W, a_self, a_neigh):
    return kernel_impl(features, A, W, a_self, a_neigh)[0]
